# revision 1
# baseline (speedup 1.0000x reference)
import numpy as np
import jax
import jax.numpy as jnp
from functools import partial

UNITS = 256
OUT_D = 512
N_CORES = 8


def _step(WaS, enc, Ua, Va, gk, grk, gb, W1, b1, W2, b2, carry, x_t):
    h, out_prev = carry
    scores = jnp.tanh(WaS + (h @ Ua)[:, None, :]) @ Va      # (b, T_enc, 1)
    e = jax.nn.softmax(scores[..., 0], axis=-1)             # (b, T_enc)
    c = jnp.einsum('bt,bte->be', e, enc)                    # (b, E)
    x = jnp.concatenate([c, out_prev], axis=-1)
    xz = x @ gk + gb[0]
    hz = h @ grk + gb[1]
    z = jax.nn.sigmoid(xz[:, :UNITS] + hz[:, :UNITS])
    r = jax.nn.sigmoid(xz[:, UNITS:2 * UNITS] + hz[:, UNITS:2 * UNITS])
    hh = jnp.tanh(xz[:, 2 * UNITS:] + r * hz[:, 2 * UNITS:])
    h_new = z * h + (1.0 - z) * hh
    pre = jnp.concatenate([x_t, h_new, c], axis=-1) @ W1 + b1
    out = jnp.where(pre > 0, pre, 0.1 * pre) @ W2 + b2
    return (h_new, out), (out, e)


@partial(jax.pmap, axis_name='x')
def _run_shard(enc, dec, h0, out0, W_a, U_a, V_a, gk, grk, gb, W1, b1, W2, b2):
    WaS = jnp.einsum('bte,ef->btf', enc, W_a)
    xs = jnp.swapaxes(dec, 0, 1)                            # (T_dec, b, D)
    step = partial(_step, WaS, enc, U_a, V_a, gk, grk, gb, W1, b1, W2, b2)
    _, (outs, es) = jax.lax.scan(step, (h0, out0), xs)
    return jnp.swapaxes(outs, 0, 1), jnp.swapaxes(es, 0, 1)


def kernel(encoder_out_seq, decoder_out_seq, hidden_state, cell_state, out_state,
           W_a, U_a, V_a, gru_kernel, gru_rec_kernel, gru_bias, W1, b1, W2, b2):
    B = encoder_out_seq.shape[0]
    bl = B // N_CORES

    def shard(a):
        return np.ascontiguousarray(np.asarray(a).reshape((N_CORES, bl) + a.shape[1:]))

    def repl(a):
        a = np.asarray(a)
        return np.broadcast_to(a, (N_CORES,) + a.shape)

    outs, es = _run_shard(
        shard(encoder_out_seq), shard(decoder_out_seq),
        shard(hidden_state), shard(out_state),
        repl(W_a), repl(U_a), repl(V_a), repl(gru_kernel), repl(gru_rec_kernel),
        repl(gru_bias), repl(W1), repl(b1), repl(W2), repl(b2))
    outs = np.asarray(outs).reshape((B,) + outs.shape[2:])
    es = np.asarray(es).reshape((B,) + es.shape[2:])
    return outs, es
